# revision 26
# baseline (speedup 1.0000x reference)
"""Trainium2 Bass kernel for nn_DeepSeekNeuralMLP (SwiGLU MLP with
Catmull-Rom-spline-reconstructed weights), tensor-parallel over 8 NeuronCores.

Strategy (Megatron-style):
  - gate/up weights [8192, 2048] sharded over the intermediate dim: core r owns
    rows [r*1024, (r+1)*1024).  down weight [2048, 8192] sharded over its input
    (intermediate) dim: core r owns columns [r*1024, (r+1)*1024).  Each core
    produces a partial output [2048, 8192] (h-major, bf16); the host sums the 8
    partials in f32 and transposes to the final [4, 2048, 2048].
  - The spline reconstruction is pure input prep (it depends only on the
    control-point vectors, not on hidden_states), so it runs on the host in
    f32 and the per-core weight shards stream to the device as bf16 — the
    device program is a single fused dense pass: for each 512-token block:
    gate/up matmuls -> silu*mul -> down matmul -> partial-output DMA.  All
    three weight shards (12 MB bf16) live in SBUF; the intermediate never
    touches DRAM.  Weight-shard DMAs are emitted in first-use order with a
    one-block lookahead so the PE pipeline never stalls on them.
  - Weight SBUF layout matches the matmul slicing: [128 = sample-within-chunk,
    chunks], where flat sample n = chunk*128 + p covers W row-major; lhsT
    tiles are stride-16 (gate/up) / stride-8 (down) column slices.
"""
import numpy as np

import ml_dtypes

import concourse.bass as bass
from concourse import bacc, tile, mybir
from concourse.bass_utils import run_bass_kernel_spmd

# ----------------------------------------------------------------------------
# static problem geometry (hardcoded; must match the reference)
# ----------------------------------------------------------------------------
HIDDEN = 2048
INTER = 8192
NTOK = 8192                    # 4 * 2048 tokens
NCORES = 8
N = INTER * HIDDEN             # samples per weight (same for all three)
NCTRL = max(16, int(N / 128.9))
NCHUNK = N // 128
CPB = NCHUNK // NCORES         # 16384 chunks per core per weight

F32 = mybir.dt.float32
BF16 = mybir.dt.bfloat16
BF16_NP = ml_dtypes.bfloat16


def _spline_static():
    """Static Catmull-Rom sampling grid: gather indices + basis weights."""
    t = np.linspace(0.0, NCTRL - 1.0, N, dtype=np.float64)
    i = np.clip(np.floor(t).astype(np.int64), 0, NCTRL - 2)
    f = (t - i).astype(np.float32)
    idx = np.stack([np.clip(i + k, 0, NCTRL - 1).astype(np.int32)
                    for k in (-1, 0, 1, 2)], axis=0)      # [4, N]
    f2 = f * f
    f3 = f2 * f
    basis = np.stack([
        0.5 * (-f + 2.0 * f2 - f3),
        0.5 * (2.0 - 5.0 * f2 + 3.0 * f3),
        0.5 * (f + 4.0 * f2 - 3.0 * f3),
        0.5 * (-f2 + f3),
    ], axis=0)                                            # [4, N] f32
    return idx, basis


_IDX, _BASIS = _spline_static()


def _down_chunklist():
    """Core r owns down-weight columns [r*1024,(r+1)*1024): chunk h*64+r*8+ib."""
    h = np.arange(HIDDEN, dtype=np.int64)
    ib = np.arange(8, dtype=np.int64)
    down = np.empty((NCORES, CPB), dtype=np.int64)
    for r in range(NCORES):
        down[r] = (h[:, None] * 64 + r * 8 + ib[None, :]).reshape(-1)
    return down


_CL_DN = _down_chunklist()


def _reconstruct(cp):
    """Host-side f32 spline reconstruction of the full flat weight [N]."""
    cp = np.asarray(cp, dtype=np.float32)
    w = _BASIS[0] * cp[_IDX[0]]
    for k in (1, 2, 3):
        w += _BASIS[k] * cp[_IDX[k]]
    return w


def _shards(gate_cp, up_cp, down_cp):
    """Per-core bf16 weight shards in device SBUF layout [128, CPB]."""
    wg = _reconstruct(gate_cp).reshape(NCHUNK, 128)
    wu = _reconstruct(up_cp).reshape(NCHUNK, 128)
    wd = _reconstruct(down_cp).reshape(NCHUNK, 128)
    out = []
    for r in range(NCORES):
        sl = slice(r * CPB, (r + 1) * CPB)
        out.append({
            "gate_w": np.ascontiguousarray(wg[sl].T).astype(BF16_NP),
            "up_w": np.ascontiguousarray(wu[sl].T).astype(BF16_NP),
            "down_w": np.ascontiguousarray(wd[_CL_DN[r]].T).astype(BF16_NP),
        })
    return out


# ----------------------------------------------------------------------------
# device program
# ----------------------------------------------------------------------------
def _build_program():
    nc = bacc.Bacc("TRN2", target_bir_lowering=False, debug=False,
                   num_devices=NCORES)

    hsT = nc.dram_tensor("hsT", [HIDDEN, NTOK], BF16, kind="ExternalInput")
    w_dram = {w: nc.dram_tensor(f"{w}_w", [128, CPB], BF16,
                                kind="ExternalInput")
              for w in ("gate", "up", "down")}
    outT = nc.dram_tensor("outT", [HIDDEN, NTOK], BF16, kind="ExternalOutput")

    with tile.TileContext(nc) as tc:
        import contextlib
        with contextlib.ExitStack() as ctx:
            pools = {
                "wgt": ctx.enter_context(tc.tile_pool(name="wgt", bufs=6)),
                "hs": ctx.enter_context(tc.tile_pool(name="hs", bufs=48)),
                "sil": ctx.enter_context(tc.tile_pool(name="sil", bufs=4)),
                "inter": ctx.enter_context(tc.tile_pool(name="inter", bufs=16)),
                "ot": ctx.enter_context(tc.tile_pool(name="ot", bufs=4)),
                "psum": ctx.enter_context(
                    tc.tile_pool(name="psum", bufs=8, space="PSUM")),
            }
            # persistent bf16 weight shards, [128 sample-in-chunk, 8192 chunks]
            # per half (half = chunk super-blocks 0..7 / 8..15)
            wt = {}
            for w in ("gate", "up", "down"):
                wt[w] = [pools["wgt"].tile([128, CPB // 2], BF16, tag="wgt",
                                           name=f"{w}_h{i}") for i in range(2)]

            def load_sb(w, sb, nsb=1):
                """DMA chunk super-blocks [sb, sb+nsb) of weight w (must not
                cross the half boundary)."""
                half, lsb = sb // 8, sb % 8
                nc.sync.dma_start(
                    wt[w][half][:, lsb * 1024:(lsb + nsb) * 1024],
                    w_dram[w][:, sb * 1024:(sb + nsb) * 1024])

            # ---- fused main loop over 512-token blocks -----------------------
            for tb in range(16):
                if tb == 0:
                    # it=0's weights as four 256 KB transfers on parallel
                    # queues (gate first — it gates the first matmul)
                    load_sb("gate", 0)
                    load_sb("gate", 1)
                    load_sb("up", 0)
                    load_sb("up", 1)
                hs_tiles = []
                for kt in range(16):
                    t = pools["hs"].tile([128, 512], BF16, tag="hs", name="hst")
                    nc.sync.dma_start(
                        t[:],
                        hsT[kt * 128:(kt + 1) * 128, tb * 512:(tb + 1) * 512])
                    hs_tiles.append(t)
                if tb == 0:
                    # it=1's weights behind the hs block
                    load_sb("gate", 2, 2)
                    load_sb("up", 2, 2)
                int_tiles = []
                for it in range(8):
                    half, lit = it // 4, it % 4
                    pg = pools["psum"].tile([128, 512], F32, tag="ps", name="pg")
                    pu = pools["psum"].tile([128, 512], F32, tag="ps", name="pu")
                    for kt in range(16):
                        base = lit * 2048 + kt
                        lg = wt["gate"][half][:, base:base + 2033:16]
                        lu = wt["up"][half][:, base:base + 2033:16]
                        rhs = hs_tiles[kt][:]
                        nc.tensor.matmul(pg[:], lg, rhs,
                                         start=(kt == 0), stop=(kt == 15))
                        nc.tensor.matmul(pu[:], lu, rhs,
                                         start=(kt == 0), stop=(kt == 15))
                    sil = pools["sil"].tile([128, 512], F32, tag="sil",
                                            name="sil")
                    nc.scalar.activation(sil[:], pg[:],
                                         mybir.ActivationFunctionType.Silu)
                    itile = pools["inter"].tile([128, 512], BF16, tag="itile",
                                                name="itile")
                    nc.vector.tensor_mul(itile[:], sil[:], pu[:])
                    int_tiles.append(itile)
                    if tb == 0:
                        # two-block lookahead for gate/up; spread the down
                        # shard 2 super-blocks per it so it lands before the
                        # down phase
                        if it < 6:
                            load_sb("gate", 2 * it + 4, 2)
                            load_sb("up", 2 * it + 4, 2)
                        load_sb("down", 2 * it, 2)
                for ht in range(16):
                    half, lht = ht // 8, ht % 8
                    pd = pools["psum"].tile([128, 512], F32, tag="ps", name="pd")
                    for it in range(8):
                        base = lht * 1024 + it
                        ld = wt["down"][half][:, base:base + 1017:8]
                        nc.tensor.matmul(pd[:], ld, int_tiles[it][:],
                                         start=(it == 0), stop=(it == 7))
                    ot = pools["ot"].tile([128, 512], BF16, tag="ot", name="ot")
                    nc.scalar.copy(ot[:], pd[:])
                    nc.sync.dma_start(
                        outT[ht * 128:(ht + 1) * 128, tb * 512:(tb + 1) * 512],
                        ot[:])

    nc.compile()
    return nc


_NC_CACHE = None


def _get_program():
    global _NC_CACHE
    if _NC_CACHE is None:
        _NC_CACHE = _build_program()
    return _NC_CACHE


def kernel(hidden_states, gate_cp, up_cp, down_cp, _trace=False):
    nc = _get_program()
    hs = np.ascontiguousarray(
        np.asarray(hidden_states, dtype=np.float32).reshape(NTOK, HIDDEN).T
    ).astype(BF16_NP)
    shards = _shards(gate_cp, up_cp, down_cp)
    maps = [{"hsT": hs, **shards[r]} for r in range(NCORES)]
    res = run_bass_kernel_spmd(nc, maps, core_ids=list(range(NCORES)),
                               trace=_trace)
    out_T = np.zeros((HIDDEN, NTOK), dtype=np.float32)
    for r in range(NCORES):
        out_T += res.results[r]["outT"].astype(np.float32)
    out = np.ascontiguousarray(out_T.T).reshape(4, 2048, HIDDEN)
    if _trace:
        kernel.last_results = res
    return out


# revision 30
# speedup vs baseline: 1.0006x; 1.0006x over previous
"""Trainium2 Bass kernel for nn_DeepSeekNeuralMLP (SwiGLU MLP with
Catmull-Rom-spline-reconstructed weights), tensor-parallel over 8 NeuronCores.

Strategy (Megatron-style):
  - gate/up weights [8192, 2048] sharded over the intermediate dim: core r owns
    rows [r*1024, (r+1)*1024).  down weight [2048, 8192] sharded over its input
    (intermediate) dim: core r owns columns [r*1024, (r+1)*1024).  Each core
    produces a partial output [2048, 8192] (h-major, bf16); the host sums the 8
    partials in f32 and transposes to the final [4, 2048, 2048].
  - The spline reconstruction is pure input prep (it depends only on the
    control-point vectors, not on hidden_states), so it runs on the host in
    f32 and the per-core weight shards stream to the device as bf16 — the
    device program is a single fused dense pass: for each 512-token block:
    gate/up matmuls -> silu*mul -> down matmul -> partial-output DMA.  All
    three weight shards (12 MB bf16) live in SBUF; the intermediate never
    touches DRAM.  Weight-shard DMAs are emitted in first-use order with a
    one-block lookahead so the PE pipeline never stalls on them.
  - Weight SBUF layout matches the matmul slicing: [128 = sample-within-chunk,
    chunks], where flat sample n = chunk*128 + p covers W row-major; lhsT
    tiles are stride-16 (gate/up) / stride-8 (down) column slices.
"""
import numpy as np

import ml_dtypes

import concourse.bass as bass
from concourse import bacc, tile, mybir
from concourse.bass_utils import run_bass_kernel_spmd

# ----------------------------------------------------------------------------
# static problem geometry (hardcoded; must match the reference)
# ----------------------------------------------------------------------------
HIDDEN = 2048
INTER = 8192
NTOK = 8192                    # 4 * 2048 tokens
NCORES = 8
N = INTER * HIDDEN             # samples per weight (same for all three)
NCTRL = max(16, int(N / 128.9))
NCHUNK = N // 128
CPB = NCHUNK // NCORES         # 16384 chunks per core per weight

F32 = mybir.dt.float32
BF16 = mybir.dt.bfloat16
BF16_NP = ml_dtypes.bfloat16


def _spline_static():
    """Static Catmull-Rom sampling grid: gather indices + basis weights."""
    t = np.linspace(0.0, NCTRL - 1.0, N, dtype=np.float64)
    i = np.clip(np.floor(t).astype(np.int64), 0, NCTRL - 2)
    f = (t - i).astype(np.float32)
    idx = np.stack([np.clip(i + k, 0, NCTRL - 1).astype(np.int32)
                    for k in (-1, 0, 1, 2)], axis=0)      # [4, N]
    f2 = f * f
    f3 = f2 * f
    basis = np.stack([
        0.5 * (-f + 2.0 * f2 - f3),
        0.5 * (2.0 - 5.0 * f2 + 3.0 * f3),
        0.5 * (f + 4.0 * f2 - 3.0 * f3),
        0.5 * (-f2 + f3),
    ], axis=0)                                            # [4, N] f32
    return idx, basis


_IDX, _BASIS = _spline_static()


def _down_chunklist():
    """Core r owns down-weight columns [r*1024,(r+1)*1024): chunk h*64+r*8+ib."""
    h = np.arange(HIDDEN, dtype=np.int64)
    ib = np.arange(8, dtype=np.int64)
    down = np.empty((NCORES, CPB), dtype=np.int64)
    for r in range(NCORES):
        down[r] = (h[:, None] * 64 + r * 8 + ib[None, :]).reshape(-1)
    return down


_CL_DN = _down_chunklist()


def _reconstruct(cp):
    """Host-side f32 spline reconstruction of the full flat weight [N]."""
    cp = np.asarray(cp, dtype=np.float32)
    w = _BASIS[0] * cp[_IDX[0]]
    for k in (1, 2, 3):
        w += _BASIS[k] * cp[_IDX[k]]
    return w


def _shards(gate_cp, up_cp, down_cp):
    """Per-core bf16 weight shards in device SBUF layout [128, CPB]."""
    wg = _reconstruct(gate_cp).reshape(NCHUNK, 128)
    wu = _reconstruct(up_cp).reshape(NCHUNK, 128)
    wd = _reconstruct(down_cp).reshape(NCHUNK, 128)
    out = []
    for r in range(NCORES):
        sl = slice(r * CPB, (r + 1) * CPB)
        out.append({
            "gate_w": np.ascontiguousarray(wg[sl].T).astype(BF16_NP),
            "up_w": np.ascontiguousarray(wu[sl].T).astype(BF16_NP),
            "down_w": np.ascontiguousarray(wd[_CL_DN[r]].T).astype(BF16_NP),
        })
    return out


# ----------------------------------------------------------------------------
# device program
# ----------------------------------------------------------------------------
def _build_program():
    nc = bacc.Bacc("TRN2", target_bir_lowering=False, debug=False,
                   num_devices=NCORES)

    hsT = nc.dram_tensor("hsT", [HIDDEN, NTOK], BF16, kind="ExternalInput")
    w_dram = {w: nc.dram_tensor(f"{w}_w", [128, CPB], BF16,
                                kind="ExternalInput")
              for w in ("gate", "up", "down")}
    warm_d = nc.dram_tensor("warm", [128, 128], BF16, kind="ExternalInput")
    outT = nc.dram_tensor("outT", [HIDDEN, NTOK], BF16, kind="ExternalOutput")

    with tile.TileContext(nc) as tc:
        import contextlib
        with contextlib.ExitStack() as ctx:
            pools = {
                "warm": ctx.enter_context(tc.tile_pool(name="warm", bufs=1)),
                "wgt": ctx.enter_context(tc.tile_pool(name="wgt", bufs=6)),
                "hs": ctx.enter_context(tc.tile_pool(name="hs", bufs=48)),
                "sil": ctx.enter_context(tc.tile_pool(name="sil", bufs=4)),
                "inter": ctx.enter_context(tc.tile_pool(name="inter", bufs=16)),
                "ot": ctx.enter_context(tc.tile_pool(name="ot", bufs=4)),
                "psum": ctx.enter_context(
                    tc.tile_pool(name="psum", bufs=8, space="PSUM")),
            }
            # persistent bf16 weight shards, [128 sample-in-chunk, 8192 chunks]
            # per half (half = chunk super-blocks 0..7 / 8..15)
            wt = {}
            for w in ("gate", "up", "down"):
                wt[w] = [pools["wgt"].tile([128, CPB // 2], BF16, tag="wgt",
                                           name=f"{w}_h{i}") for i in range(2)]

            # HAM warm-up: dependency-free dummy matmuls fill the otherwise
            # PE-idle DMA ramp, flipping the clock gate to 8/8 before the
            # first real matmul's data lands (~48 x 107ns cold ends ~9us,
            # ahead of the ~13.7us first real matmul)
            warm_t = pools["warm"].tile([128, 128], BF16, tag="warm",
                                        name="warm")
            nc.sync.dma_start(warm_t[:], warm_d[:])
            pw = pools["psum"].tile([128, 128], F32, tag="ps", name="pw")
            for _ in range(48):
                nc.tensor.matmul(pw[:], warm_t[:], warm_t[:],
                                 start=True, stop=True)

            def load_sb(w, sb, nsb=1):
                """DMA chunk super-blocks [sb, sb+nsb) of weight w (must not
                cross the half boundary)."""
                half, lsb = sb // 8, sb % 8
                nc.sync.dma_start(
                    wt[w][half][:, lsb * 1024:(lsb + nsb) * 1024],
                    w_dram[w][:, sb * 1024:(sb + nsb) * 1024])

            # ---- fused main loop over 512-token blocks -----------------------
            for tb in range(16):
                if tb == 0:
                    # it=0's weights as four 256 KB transfers on parallel
                    # queues (gate first — it gates the first matmul)
                    load_sb("gate", 0)
                    load_sb("gate", 1)
                    load_sb("up", 0)
                    load_sb("up", 1)
                hs_tiles = []
                for kt in range(16):
                    t = pools["hs"].tile([128, 512], BF16, tag="hs", name="hst")
                    nc.sync.dma_start(
                        t[:],
                        hsT[kt * 128:(kt + 1) * 128, tb * 512:(tb + 1) * 512])
                    hs_tiles.append(t)
                if tb == 0:
                    # it=1's weights behind the hs block
                    load_sb("gate", 2, 2)
                    load_sb("up", 2, 2)
                int_tiles = []
                for it in range(8):
                    half, lit = it // 4, it % 4
                    pg = pools["psum"].tile([128, 512], F32, tag="ps", name="pg")
                    pu = pools["psum"].tile([128, 512], F32, tag="ps", name="pu")
                    for kt in range(16):
                        base = lit * 2048 + kt
                        lg = wt["gate"][half][:, base:base + 2033:16]
                        lu = wt["up"][half][:, base:base + 2033:16]
                        rhs = hs_tiles[kt][:]
                        nc.tensor.matmul(pg[:], lg, rhs,
                                         start=(kt == 0), stop=(kt == 15))
                        nc.tensor.matmul(pu[:], lu, rhs,
                                         start=(kt == 0), stop=(kt == 15))
                    sil = pools["sil"].tile([128, 512], F32, tag="sil",
                                            name="sil")
                    nc.scalar.activation(sil[:], pg[:],
                                         mybir.ActivationFunctionType.Silu)
                    itile = pools["inter"].tile([128, 512], BF16, tag="itile",
                                                name="itile")
                    nc.vector.tensor_mul(itile[:], sil[:], pu[:])
                    int_tiles.append(itile)
                    if tb == 0:
                        # two-block lookahead for gate/up; spread the down
                        # shard 2 super-blocks per it so it lands before the
                        # down phase
                        if it < 6:
                            load_sb("gate", 2 * it + 4, 2)
                            load_sb("up", 2 * it + 4, 2)
                        load_sb("down", 2 * it, 2)
                for ht in range(16):
                    half, lht = ht // 8, ht % 8
                    pd = pools["psum"].tile([128, 512], F32, tag="ps", name="pd")
                    for it in range(8):
                        base = lht * 1024 + it
                        ld = wt["down"][half][:, base:base + 1017:8]
                        nc.tensor.matmul(pd[:], ld, int_tiles[it][:],
                                         start=(it == 0), stop=(it == 7))
                    ot = pools["ot"].tile([128, 512], BF16, tag="ot", name="ot")
                    nc.scalar.copy(ot[:], pd[:])
                    nc.sync.dma_start(
                        outT[ht * 128:(ht + 1) * 128, tb * 512:(tb + 1) * 512],
                        ot[:])

    nc.compile()
    return nc


_NC_CACHE = None


def _get_program():
    global _NC_CACHE
    if _NC_CACHE is None:
        _NC_CACHE = _build_program()
    return _NC_CACHE


def kernel(hidden_states, gate_cp, up_cp, down_cp, _trace=False):
    nc = _get_program()
    hs = np.ascontiguousarray(
        np.asarray(hidden_states, dtype=np.float32).reshape(NTOK, HIDDEN).T
    ).astype(BF16_NP)
    shards = _shards(gate_cp, up_cp, down_cp)
    warm = np.zeros((128, 128), dtype=BF16_NP)
    maps = [{"hsT": hs, "warm": warm, **shards[r]} for r in range(NCORES)]
    res = run_bass_kernel_spmd(nc, maps, core_ids=list(range(NCORES)),
                               trace=_trace)
    out_T = np.zeros((HIDDEN, NTOK), dtype=np.float32)
    for r in range(NCORES):
        out_T += res.results[r]["outT"].astype(np.float32)
    out = np.ascontiguousarray(out_T.T).reshape(4, 2048, HIDDEN)
    if _trace:
        kernel.last_results = res
    return out
